# revision 11
# baseline (speedup 1.0000x reference)
"""Llama-style 2-layer transformer forward on 8 Trainium2 NeuronCores.

Sharding: sequence-parallel. Each core owns S/8 = 256 tokens of the residual
stream. Weights are replicated (bf16, RMSNorm scales folded in, pre-arranged
K-major on host). Per layer, each core computes q/k/v for its own tokens,
ropes them, and the full K/V (all 2048 tokens) is assembled with one
AllGather; attention/MLP are then entirely local to the core's 256 queries.
Final output shards are concatenated on host.
"""

import os
import numpy as np
import ml_dtypes

import concourse.bass as bass
import concourse.tile as tile
from concourse import bacc, mybir
from concourse import bass_utils
from concourse.bass import ds
from concourse.masks import make_identity

P = 128
B, S, H, NH, NKV, L, I, V = 1, 2048, 2048, 16, 8, 2, 8192, 32000
HD = H // NH            # 128
REP = NH // NKV         # 2
NCORES = 8
TC = S // NCORES        # 256 tokens per core
NJ = TC // P            # 2 token chunks per core
KS = H // P             # 16 contraction subtiles over H
DS = I // P             # 64 subtiles over I
EPS = 1e-5
THETA = 10000.0
SCALE = HD ** -0.5
SK = S // P             # 16 key chunks

BF = mybir.dt.bfloat16
F32 = mybir.dt.float32
I32 = mybir.dt.int32
AF = mybir.ActivationFunctionType
OP = mybir.AluOpType

LAST_RESULT = None  # stashed BassKernelResults for test harness introspection


def _build():
    nc = bacc.Bacc("TRN2", target_bir_lowering=False, debug=False,
                   enable_asserts=False, num_devices=NCORES)

    ids_ap = nc.dram_tensor("ids", [TC], I32, kind="ExternalInput").ap()
    emb_ap = nc.dram_tensor("embed", [V, H], BF, kind="ExternalInput").ap()
    wq_ap = nc.dram_tensor("wq", [L, P, KS, H], BF, kind="ExternalInput").ap()
    wk_ap = nc.dram_tensor("wk", [L, P, KS, NKV * HD], BF, kind="ExternalInput").ap()
    wv_ap = nc.dram_tensor("wv", [L, P, KS, NKV * HD], BF, kind="ExternalInput").ap()
    wo_ap = nc.dram_tensor("wo", [L, P, KS, H], BF, kind="ExternalInput").ap()
    wg_ap = nc.dram_tensor("wg", [L, P, DS, KS, P], BF, kind="ExternalInput").ap()
    wu_ap = nc.dram_tensor("wu", [L, P, DS, KS, P], BF, kind="ExternalInput").ap()
    wd_ap = nc.dram_tensor("wd", [L, P, DS, H], BF, kind="ExternalInput").ap()
    cos_ap = nc.dram_tensor("cos", [P, NJ, HD], F32, kind="ExternalInput").ap()
    sin_ap = nc.dram_tensor("sin", [P, NJ, HD], F32, kind="ExternalInput").ap()
    nw_ap = nc.dram_tensor("normw", [P, H], F32, kind="ExternalInput").ap()
    out_ap = nc.dram_tensor("out", [TC, H], F32, kind="ExternalOutput").ap()

    with tile.TileContext(nc) as tc:
        with (
            tc.tile_pool(name="const", bufs=1) as const,
            tc.tile_pool(name="xres", bufs=1) as xpool,
            tc.tile_pool(name="acts", bufs=1) as acts,
            tc.tile_pool(name="wstream", bufs=2) as sbw,
            tc.tile_pool(name="wsmall", bufs=2) as sbw3,
            tc.tile_pool(name="scr", bufs=2) as scr,
            tc.tile_pool(name="ps_big", bufs=1, space="PSUM") as ps_big,
            tc.tile_pool(name="ps_t", bufs=2, space="PSUM") as ps_t,
            tc.tile_pool(name="ps_mm", bufs=2, space="PSUM") as ps_mm,
            tc.tile_pool(name="dram", bufs=1, space="DRAM") as dram,
        ):
            ident = const.tile([P, P], BF)
            make_identity(nc, ident[:])
            cos_sb = const.tile([P, NJ, HD], F32)
            nc.sync.dma_start(cos_sb[:], cos_ap[:])
            sin_sb = const.tile([P, NJ, HD], F32)
            nc.sync.dma_start(sin_sb[:], sin_ap[:])
            nw_sb = const.tile([P, H], F32)
            nc.sync.dma_start(nw_sb[:], nw_ap[:])

            x = xpool.tile([P, NJ, H], F32)

            # ---- embedding gather ----
            for j in range(NJ):
                ids_sb = scr.tile([P, 1], I32, tag="ids")
                nc.sync.dma_start(ids_sb[:], ids_ap[ds(j * P, P), None])
                erows = scr.tile([P, H], BF, tag="halfb", bufs=1)
                nc.gpsimd.indirect_dma_start(
                    out=erows[:], out_offset=None, in_=emb_ap[:],
                    in_offset=bass.IndirectOffsetOnAxis(ap=ids_sb[:, :1], axis=0),
                )
                nc.any.tensor_copy(x[:, j, :], erows[:])

            def rmsnorm_T():
                """x -> normed, transposed [P, KS, TC] bf16 (ln scale folded into W)."""
                hT = acts.tile([P, KS, TC], BF, tag="med1", bufs=2)
                for j in range(NJ):
                    sq = ps_big.tile([P, S], F32, tag="sc")
                    ssum = scr.tile([P, 1], F32, tag="ssum")
                    nc.scalar.activation(sq[:], x[:, j, :], AF.Square,
                                         accum_out=ssum[:])
                    var = scr.tile([P, 1], F32, tag="var")
                    nc.vector.tensor_scalar(var[:], ssum[:], 1.0 / H, EPS,
                                            OP.mult, OP.add)
                    rec = scr.tile([P, 1], F32, tag="rec")
                    nc.vector.reciprocal(rec[:], var[:])
                    rstd = scr.tile([P, 1], F32, tag="rstd")
                    nc.scalar.activation(rstd[:], rec[:], AF.Sqrt)
                    hn = scr.tile([P, H], BF, tag="halfb", bufs=1)
                    nc.vector.tensor_scalar_mul(hn[:], x[:, j, :], rstd[:, :1])
                    for ks in range(KS):
                        pt = ps_t.tile([P, P], BF, tag="pt")
                        nc.tensor.transpose(pt[:], hn[:, ds(ks * P, P)], ident[:])
                        nc.any.tensor_copy(hT[:, ks, ds(j * P, P)], pt[:])
                return hT

            def rope(src, nh, j, dst):
                """src [P, nh, HD] bf16 token-major -> dst bf16; dst may be src."""
                t2 = scr.tile([P, NH, HD], BF, tag="ropet", bufs=1)
                t2 = t2[:, :nh, :]
                h_ = HD // 2
                # t2 = rotate_half(src) * sin, computed before src is clobbered
                nc.vector.tensor_tensor(
                    t2[:, :, 0:h_], src[:, :, h_:HD],
                    sin_sb[:, j, None, 0:h_].to_broadcast([P, nh, h_]), OP.mult)
                nc.vector.tensor_tensor(
                    t2[:, :, h_:HD], src[:, :, 0:h_],
                    sin_sb[:, j, None, h_:HD].to_broadcast([P, nh, h_]), OP.mult)
                nc.vector.tensor_tensor(
                    dst[:], src[:],
                    cos_sb[:, j, None, :].to_broadcast([P, nh, HD]), OP.mult)
                nc.vector.tensor_tensor(dst[:, :, 0:h_], dst[:, :, 0:h_],
                                        t2[:, :, 0:h_], OP.subtract)
                nc.vector.tensor_tensor(dst[:, :, h_:HD], dst[:, :, h_:HD],
                                        t2[:, :, h_:HD], OP.add)

            for l in range(L):
                # ======== attention ========
                hT = rmsnorm_T()
                qT = acts.tile([P, NH, TC], BF, tag="med1", bufs=2)
                ag_in = dram.tile([TC, 2 * NKV * HD], BF, tag="agin")
                for j in range(NJ):
                    # --- q proj (token-major) ---
                    q_sb = scr.tile([P, NH, HD], BF, tag="q_sb", bufs=1)
                    for n in range(4):
                        wt = sbw.tile([P, KS, 512], BF, tag="wstream")
                        nc.sync.dma_start(wt[:], wq_ap[l][:, :, ds(n * 512, 512)])
                        qp = ps_mm.tile([P, 4, P], F32, tag="mm")
                        for ks in range(KS):
                            nc.tensor.matmul(qp[:], lhsT=hT[:, ks, ds(j * P, P)],
                                             rhs=wt[:, ks, :],
                                             start=(ks == 0), stop=(ks == KS - 1))
                        nc.any.tensor_copy(q_sb[:, ds(n * 4, 4), :], qp[:])
                    rope(q_sb, NH, j, q_sb)
                    for h in range(NH):
                        pt = ps_t.tile([P, P], BF, tag="pt")
                        nc.tensor.transpose(pt[:], q_sb[:, h, :], ident[:])
                        nc.any.tensor_copy(qT[:, h, ds(j * P, P)], pt[:])
                    # --- k,v proj ---
                    kv_stage = scr.tile([P, 2 * NKV, HD], BF, tag="kvst", bufs=1)
                    k_sb = scr.tile([P, NKV, HD], BF, tag="k_sb", bufs=1)
                    for n in range(2):
                        wt = sbw.tile([P, KS, 512], BF, tag="wstream")
                        nc.sync.dma_start(wt[:], wk_ap[l][:, :, ds(n * 512, 512)])
                        kp = ps_mm.tile([P, 4, P], F32, tag="mm")
                        for ks in range(KS):
                            nc.tensor.matmul(kp[:], lhsT=hT[:, ks, ds(j * P, P)],
                                             rhs=wt[:, ks, :],
                                             start=(ks == 0), stop=(ks == KS - 1))
                        nc.any.tensor_copy(k_sb[:, ds(n * 4, 4), :], kp[:])
                    for n in range(2):
                        wt = sbw.tile([P, KS, 512], BF, tag="wstream")
                        nc.sync.dma_start(wt[:], wv_ap[l][:, :, ds(n * 512, 512)])
                        vp = ps_mm.tile([P, 4, P], F32, tag="mm")
                        for ks in range(KS):
                            nc.tensor.matmul(vp[:], lhsT=hT[:, ks, ds(j * P, P)],
                                             rhs=wt[:, ks, :],
                                             start=(ks == 0), stop=(ks == KS - 1))
                        nc.any.tensor_copy(kv_stage[:, ds(NKV + n * 4, 4), :], vp[:])
                    rope(k_sb, NKV, j, kv_stage[:, 0:NKV, :])
                    nc.sync.dma_start(ag_in[ds(j * P, P), :], kv_stage[:])

                ag_out = dram.tile([S, 2 * NKV * HD], BF, tag="agout",
                                   addr_space="Shared")
                nc.gpsimd.collective_compute(
                    "AllGather", OP.bypass,
                    replica_groups=[list(range(NCORES))],
                    ins=[ag_in.opt()], outs=[ag_out.opt()],
                )

                oT = acts.tile([P, NH, TC], BF, tag="med1", bufs=2)
                for kv8 in range(NKV):
                    kTt = acts.tile([P, SK, P], BF, tag="kTt", bufs=2)
                    for sk in range(SK):
                        kk = scr.tile([P, P], BF, tag="kk", bufs=2)
                        nc.sync.dma_start(kk[:], ag_out[ds(sk * P, P),
                                                        ds(kv8 * HD, HD)])
                        pt = ps_t.tile([P, P], BF, tag="pt")
                        nc.tensor.transpose(pt[:], kk[:], ident[:])
                        nc.any.tensor_copy(kTt[:, sk, :], pt[:])
                    vt = acts.tile([P, SK, P], BF, tag="vt", bufs=2)
                    for sk in range(SK):
                        nc.sync.dma_start(vt[:, sk, :],
                                          ag_out[ds(sk * P, P),
                                                 ds(NKV * HD + kv8 * HD, HD)])
                    for h in (2 * kv8, 2 * kv8 + 1):
                        attT = scr.tile([P, SK, TC], BF, tag="attT", bufs=1)
                        for j in range(NJ):
                            sc = ps_big.tile([P, S], F32, tag="sc")
                            for n in range(4):
                                nc.tensor.matmul(
                                    sc[:, ds(n * 512, 512)],
                                    lhsT=qT[:, h, ds(j * P, P)],
                                    rhs=kTt[:, ds(n * 4, 4), :],
                                    start=True, stop=True)
                            att = scr.tile([P, S], BF, tag="att", bufs=1)
                            rsum = scr.tile([P, 1], F32, tag="rsum")
                            nc.scalar.activation(att[:], sc[:], AF.Exp,
                                                 scale=SCALE, accum_out=rsum[:])
                            rrec = scr.tile([P, 1], F32, tag="rrec")
                            nc.vector.reciprocal(rrec[:], rsum[:])
                            nc.vector.tensor_scalar_mul(att[:], att[:], rrec[:, :1])
                            for sk in range(SK):
                                pt = ps_t.tile([P, P], BF, tag="pt")
                                nc.tensor.transpose(pt[:], att[:, ds(sk * P, P)],
                                                    ident[:])
                                nc.any.tensor_copy(attT[:, sk, ds(j * P, P)], pt[:])
                        op_ = ps_mm.tile([P, TC], F32, tag="mm")
                        for sk in range(SK):
                            nc.tensor.matmul(op_[:], lhsT=vt[:, sk, :],
                                             rhs=attT[:, sk, :],
                                             start=(sk == 0), stop=(sk == SK - 1))
                        nc.any.tensor_copy(oT[:, h, :], op_[:])

                # --- o proj + residual ---
                for n in range(4):
                    wt = sbw.tile([P, KS, 512], BF, tag="wstream")
                    nc.sync.dma_start(wt[:], wo_ap[l][:, :, ds(n * 512, 512)])
                    for jt in range(NJ):
                        pp = ps_mm.tile([P, 512], F32, tag="mm")
                        for ks in range(KS):
                            nc.tensor.matmul(pp[:], lhsT=oT[:, ks, ds(jt * P, P)],
                                             rhs=wt[:, ks, :],
                                             start=(ks == 0), stop=(ks == KS - 1))
                        nc.vector.tensor_add(x[:, jt, ds(n * 512, 512)],
                                             x[:, jt, ds(n * 512, 512)], pp[:])

                # ======== MLP ========
                # I split into 4 quarters of 2048 to bound SBUF for the
                # silu(g)*u tensor; down-proj accumulates partials into x.
                h2T = rmsnorm_T()
                QD = DS // 4  # 16 I-subtiles per quarter
                for qtr in range(4):
                    act_sb = acts.tile([P, QD, TC], BF, tag="act_sb", bufs=1)
                    for icq in range(QD):
                        ic = qtr * QD + icq
                        wgt = sbw3.tile([P, KS, P], BF, tag="wg")
                        nc.sync.dma_start(wgt[:], wg_ap[l][:, ic, :, :])
                        wut = sbw3.tile([P, KS, P], BF, tag="wu")
                        nc.sync.dma_start(wut[:], wu_ap[l][:, ic, :, :])
                        gp = ps_mm.tile([P, TC], F32, tag="mm")
                        for ks in range(KS):
                            nc.tensor.matmul(gp[:], lhsT=wgt[:, ks, :],
                                             rhs=h2T[:, ks, :],
                                             start=(ks == 0), stop=(ks == KS - 1))
                        up = ps_big.tile([P, TC], F32, tag="sc")
                        for ks in range(KS):
                            nc.tensor.matmul(up[:], lhsT=wut[:, ks, :],
                                             rhs=h2T[:, ks, :],
                                             start=(ks == 0), stop=(ks == KS - 1))
                        gs = scr.tile([P, TC], F32, tag="gs")
                        nc.scalar.activation(gs[:], gp[:], AF.Silu)
                        nc.vector.tensor_tensor(act_sb[:, icq, :], gs[:], up[:],
                                                OP.mult)
                    # partial down proj for this quarter
                    for n in range(8):
                        wt = sbw.tile([P, QD, 256], BF, tag="wstream")
                        nc.sync.dma_start(
                            wt[:], wd_ap[l][:, ds(qtr * QD, QD), ds(n * 256, 256)])
                        for jt in range(NJ):
                            pp = ps_mm.tile([P, 256], F32, tag="mm")
                            for ks in range(QD):
                                nc.tensor.matmul(pp[:],
                                                 lhsT=act_sb[:, ks, ds(jt * P, P)],
                                                 rhs=wt[:, ks, :],
                                                 start=(ks == 0),
                                                 stop=(ks == QD - 1))
                            nc.vector.tensor_add(x[:, jt, ds(n * 256, 256)],
                                                 x[:, jt, ds(n * 256, 256)], pp[:])

            # ---- final rmsnorm ----
            for j in range(NJ):
                sq = ps_big.tile([P, S], F32, tag="sc")
                ssum = scr.tile([P, 1], F32, tag="ssum")
                nc.scalar.activation(sq[:], x[:, j, :], AF.Square, accum_out=ssum[:])
                var = scr.tile([P, 1], F32, tag="var")
                nc.vector.tensor_scalar(var[:], ssum[:], 1.0 / H, EPS, OP.mult, OP.add)
                rec = scr.tile([P, 1], F32, tag="rec")
                nc.vector.reciprocal(rec[:], var[:])
                rstd = scr.tile([P, 1], F32, tag="rstd")
                nc.scalar.activation(rstd[:], rec[:], AF.Sqrt)
                fin = scr.tile([P, H], F32, tag="fin", bufs=1)
                nc.vector.tensor_scalar_mul(fin[:], x[:, j, :], rstd[:, :1])
                nc.vector.tensor_tensor(fin[:], fin[:], nw_sb[:], OP.mult)
                nc.sync.dma_start(out_ap[ds(j * P, P), :], fin[:])

    nc.compile()
    return nc


def _prep_inputs(input_ids, embed, Wq, Wk, Wv, Wo, Wg, Wu, Wd, ln1, ln2, norm_w):
    bf16 = ml_dtypes.bfloat16
    f32 = np.float32
    ids = np.asarray(input_ids).reshape(S).astype(np.int32)
    embed = np.ascontiguousarray(np.asarray(embed, f32).astype(bf16))

    ln1 = np.asarray(ln1, f32)
    ln2 = np.asarray(ln2, f32)

    def kmajor(w, ks):
        w = np.asarray(w, f32)
        Ld, K, N = w.shape
        return np.ascontiguousarray(
            w.reshape(Ld, ks, P, N).transpose(0, 2, 1, 3).astype(bf16))

    wq = kmajor(np.asarray(Wq, f32) * ln1[:, :, None], KS)
    wk = kmajor(np.asarray(Wk, f32) * ln1[:, :, None], KS)
    wv = kmajor(np.asarray(Wv, f32) * ln1[:, :, None], KS)
    wo = kmajor(np.asarray(Wo, f32), KS)
    wd = kmajor(np.asarray(Wd, f32), DS)

    def gmajor(w):
        w = np.asarray(w, f32) * ln2[:, :, None]
        # [L, H, I] -> [L, kp=128, ic=64, ks=16, col=128]
        return np.ascontiguousarray(
            w.reshape(L, KS, P, DS, P).transpose(0, 2, 3, 1, 4).astype(bf16))

    wg = gmajor(Wg)
    wu = gmajor(Wu)

    inv = 1.0 / (THETA ** (np.arange(0, HD, 2, dtype=np.float64) / HD))  # [64]
    pos = np.arange(S, dtype=np.float64)
    fr = pos[:, None] * inv[None, :]
    cosf = np.concatenate([np.cos(fr), np.cos(fr)], -1).astype(f32)  # [S, 128]
    sinf = np.concatenate([np.sin(fr), np.sin(fr)], -1).astype(f32)

    nw = np.ascontiguousarray(np.tile(np.asarray(norm_w, f32)[None, :], (P, 1)))

    in_maps = []
    for c in range(NCORES):
        lo = c * TC
        cc = np.ascontiguousarray(
            cosf[lo:lo + TC].reshape(NJ, P, HD).transpose(1, 0, 2))
        sc_ = np.ascontiguousarray(
            sinf[lo:lo + TC].reshape(NJ, P, HD).transpose(1, 0, 2))
        in_maps.append(dict(
            ids=ids[lo:lo + TC].copy(), embed=embed,
            wq=wq, wk=wk, wv=wv, wo=wo, wg=wg, wu=wu, wd=wd,
            cos=cc, sin=sc_, normw=nw,
        ))
    return in_maps


LAST_NC = None
LAST_IN_MAPS = None


def kernel(**inputs):
    global LAST_RESULT, LAST_NC, LAST_IN_MAPS
    in_maps = _prep_inputs(**inputs)
    nc = _build()
    res = bass_utils.run_bass_kernel_spmd(nc, in_maps, core_ids=list(range(NCORES)))
    LAST_RESULT = res
    LAST_NC = nc
    LAST_IN_MAPS = in_maps
    out = np.concatenate([np.asarray(res.results[c]["out"], np.float32)
                          for c in range(NCORES)], axis=0)
    return out.reshape(B, S, H)


# revision 13
# speedup vs baseline: 422.3357x; 422.3357x over previous
"""Llama-style 2-layer transformer forward on 8 Trainium2 NeuronCores.

Sharding: sequence-parallel. Each core owns S/8 = 256 tokens of the residual
stream. Weights are replicated (bf16, RMSNorm scales folded in, pre-arranged
K-major on host). Per layer, each core computes q/k/v for its own tokens,
ropes them, and the full K/V (all 2048 tokens) is assembled with one
AllGather; attention/MLP are then entirely local to the core's 256 queries.
Final output shards are concatenated on host.
"""

import os
import numpy as np
import ml_dtypes

import concourse.bass as bass
import concourse.tile as tile
from concourse import bacc, mybir
from concourse import bass_utils
from concourse.bass import ds
from concourse.masks import make_identity

P = 128
B, S, H, NH, NKV, L, I, V = 1, 2048, 2048, 16, 8, 2, 8192, 32000
HD = H // NH            # 128
REP = NH // NKV         # 2
NCORES = 8
TC = S // NCORES        # 256 tokens per core
NJ = TC // P            # 2 token chunks per core
KS = H // P             # 16 contraction subtiles over H
DS = I // P             # 64 subtiles over I
EPS = 1e-5
THETA = 10000.0
SCALE = HD ** -0.5
SK = S // P             # 16 key chunks

BF = mybir.dt.bfloat16
F32 = mybir.dt.float32
I32 = mybir.dt.int32
AF = mybir.ActivationFunctionType
OP = mybir.AluOpType

LAST_RESULT = None  # stashed BassKernelResults for test harness introspection


def _build():
    nc = bacc.Bacc("TRN2", target_bir_lowering=False, debug=False,
                   enable_asserts=False, num_devices=NCORES)

    ids_ap = nc.dram_tensor("ids", [TC], I32, kind="ExternalInput").ap()
    emb_ap = nc.dram_tensor("embed", [V, H], BF, kind="ExternalInput").ap()
    wq_ap = nc.dram_tensor("wq", [L, P, KS, H], BF, kind="ExternalInput").ap()
    wk_ap = nc.dram_tensor("wk", [L, P, KS, NKV * HD], BF, kind="ExternalInput").ap()
    wv_ap = nc.dram_tensor("wv", [L, P, KS, NKV * HD], BF, kind="ExternalInput").ap()
    wo_ap = nc.dram_tensor("wo", [L, P, KS, H], BF, kind="ExternalInput").ap()
    wg_ap = nc.dram_tensor("wg", [L, P, DS, KS, P], BF, kind="ExternalInput").ap()
    wu_ap = nc.dram_tensor("wu", [L, P, DS, KS, P], BF, kind="ExternalInput").ap()
    wd_ap = nc.dram_tensor("wd", [L, P, DS, H], BF, kind="ExternalInput").ap()
    cos_ap = nc.dram_tensor("cos", [P, NJ, HD], F32, kind="ExternalInput").ap()
    sin_ap = nc.dram_tensor("sin", [P, NJ, HD], F32, kind="ExternalInput").ap()
    nw_ap = nc.dram_tensor("normw", [P, H], F32, kind="ExternalInput").ap()
    out_ap = nc.dram_tensor("out", [TC, H], F32, kind="ExternalOutput").ap()

    with tile.TileContext(nc) as tc:
        with (
            tc.tile_pool(name="const", bufs=1) as const,
            tc.tile_pool(name="xres", bufs=1) as xpool,
            tc.tile_pool(name="acts", bufs=1) as acts,
            tc.tile_pool(name="wstream", bufs=2) as sbw,
            tc.tile_pool(name="wsmall", bufs=2) as sbw3,
            tc.tile_pool(name="scr", bufs=2) as scr,
            tc.tile_pool(name="ps_big", bufs=1, space="PSUM") as ps_big,
            tc.tile_pool(name="ps_t", bufs=2, space="PSUM") as ps_t,
            tc.tile_pool(name="ps_mm", bufs=2, space="PSUM") as ps_mm,
            tc.tile_pool(name="dram", bufs=1, space="DRAM") as dram,
        ):
            ident = const.tile([P, P], BF)
            make_identity(nc, ident[:])
            cos_sb = const.tile([P, NJ, HD], F32)
            nc.sync.dma_start(cos_sb[:], cos_ap[:])
            sin_sb = const.tile([P, NJ, HD], F32)
            nc.sync.dma_start(sin_sb[:], sin_ap[:])
            nw_sb = const.tile([P, H], F32)
            nc.sync.dma_start(nw_sb[:], nw_ap[:])

            x = xpool.tile([P, NJ, H], F32)

            # ---- embedding gather ----
            for j in range(NJ):
                ids_sb = scr.tile([P, 1], I32, tag="ids")
                nc.sync.dma_start(ids_sb[:], ids_ap[ds(j * P, P), None])
                erows = scr.tile([P, H], BF, tag="halfb", bufs=1)
                nc.gpsimd.indirect_dma_start(
                    out=erows[:], out_offset=None, in_=emb_ap[:],
                    in_offset=bass.IndirectOffsetOnAxis(ap=ids_sb[:, :1], axis=0),
                )
                nc.vector.tensor_copy(x[:, j, :], erows[:])

            def rmsnorm_T():
                """x -> normed, transposed [P, KS, TC] bf16 (ln scale folded into W)."""
                hT = acts.tile([P, KS, TC], BF, tag="med1", bufs=2)
                for j in range(NJ):
                    sq = ps_big.tile([P, S], F32, tag="sc")
                    ssum = scr.tile([P, 1], F32, tag="ssum")
                    nc.scalar.activation(sq[:], x[:, j, :], AF.Square,
                                         accum_out=ssum[:])
                    var = scr.tile([P, 1], F32, tag="var")
                    nc.vector.tensor_scalar(var[:], ssum[:], 1.0 / H, EPS,
                                            OP.mult, OP.add)
                    rec = scr.tile([P, 1], F32, tag="rec")
                    nc.vector.reciprocal(rec[:], var[:])
                    rstd = scr.tile([P, 1], F32, tag="rstd")
                    nc.scalar.activation(rstd[:], rec[:], AF.Sqrt)
                    hn = scr.tile([P, H], BF, tag="halfb", bufs=1)
                    nc.vector.tensor_scalar_mul(hn[:], x[:, j, :], rstd[:, :1])
                    for ks in range(KS):
                        pt = ps_t.tile([P, P], BF, tag="pt")
                        nc.tensor.transpose(pt[:], hn[:, ds(ks * P, P)], ident[:])
                        nc.vector.tensor_copy(hT[:, ks, ds(j * P, P)], pt[:])
                return hT

            def rope(src, nh, j, dst):
                """src [P, nh, HD] bf16 token-major -> dst bf16; dst may be src."""
                t2 = scr.tile([P, NH, HD], BF, tag="ropet", bufs=1)
                t2 = t2[:, :nh, :]
                h_ = HD // 2
                # t2 = rotate_half(src) * sin, computed before src is clobbered
                nc.vector.tensor_tensor(
                    t2[:, :, 0:h_], src[:, :, h_:HD],
                    sin_sb[:, j, None, 0:h_].to_broadcast([P, nh, h_]), OP.mult)
                nc.vector.tensor_tensor(
                    t2[:, :, h_:HD], src[:, :, 0:h_],
                    sin_sb[:, j, None, h_:HD].to_broadcast([P, nh, h_]), OP.mult)
                nc.vector.tensor_tensor(
                    dst[:], src[:],
                    cos_sb[:, j, None, :].to_broadcast([P, nh, HD]), OP.mult)
                nc.vector.tensor_tensor(dst[:, :, 0:h_], dst[:, :, 0:h_],
                                        t2[:, :, 0:h_], OP.subtract)
                nc.vector.tensor_tensor(dst[:, :, h_:HD], dst[:, :, h_:HD],
                                        t2[:, :, h_:HD], OP.add)

            for l in range(L):
                # ======== attention ========
                hT = rmsnorm_T()
                qT = acts.tile([P, NH, TC], BF, tag="med1", bufs=2)
                ag_in = dram.tile([TC, 2 * NKV * HD], BF, tag="agin")
                for j in range(NJ):
                    # --- q proj (token-major) ---
                    q_sb = scr.tile([P, NH, HD], BF, tag="q_sb", bufs=1)
                    for n in range(4):
                        wt = sbw.tile([P, KS, 512], BF, tag="wstream")
                        nc.sync.dma_start(wt[:], wq_ap[l][:, :, ds(n * 512, 512)])
                        qp = ps_mm.tile([P, 4, P], F32, tag="mm")
                        for ks in range(KS):
                            nc.tensor.matmul(qp[:], lhsT=hT[:, ks, ds(j * P, P)],
                                             rhs=wt[:, ks, :],
                                             start=(ks == 0), stop=(ks == KS - 1))
                        nc.vector.tensor_copy(q_sb[:, ds(n * 4, 4), :], qp[:])
                    rope(q_sb, NH, j, q_sb)
                    for h in range(NH):
                        pt = ps_t.tile([P, P], BF, tag="pt")
                        nc.tensor.transpose(pt[:], q_sb[:, h, :], ident[:])
                        nc.vector.tensor_copy(qT[:, h, ds(j * P, P)], pt[:])
                    # --- k,v proj ---
                    kv_stage = scr.tile([P, 2 * NKV, HD], BF, tag="kvst", bufs=1)
                    k_sb = scr.tile([P, NKV, HD], BF, tag="k_sb", bufs=1)
                    for n in range(2):
                        wt = sbw.tile([P, KS, 512], BF, tag="wstream")
                        nc.sync.dma_start(wt[:], wk_ap[l][:, :, ds(n * 512, 512)])
                        kp = ps_mm.tile([P, 4, P], F32, tag="mm")
                        for ks in range(KS):
                            nc.tensor.matmul(kp[:], lhsT=hT[:, ks, ds(j * P, P)],
                                             rhs=wt[:, ks, :],
                                             start=(ks == 0), stop=(ks == KS - 1))
                        nc.vector.tensor_copy(k_sb[:, ds(n * 4, 4), :], kp[:])
                    for n in range(2):
                        wt = sbw.tile([P, KS, 512], BF, tag="wstream")
                        nc.sync.dma_start(wt[:], wv_ap[l][:, :, ds(n * 512, 512)])
                        vp = ps_mm.tile([P, 4, P], F32, tag="mm")
                        for ks in range(KS):
                            nc.tensor.matmul(vp[:], lhsT=hT[:, ks, ds(j * P, P)],
                                             rhs=wt[:, ks, :],
                                             start=(ks == 0), stop=(ks == KS - 1))
                        nc.vector.tensor_copy(kv_stage[:, ds(NKV + n * 4, 4), :], vp[:])
                    rope(k_sb, NKV, j, kv_stage[:, 0:NKV, :])
                    nc.sync.dma_start(ag_in[ds(j * P, P), :], kv_stage[:])

                ag_out = dram.tile([S, 2 * NKV * HD], BF, tag="agout",
                                   addr_space="Shared")
                nc.gpsimd.collective_compute(
                    "AllGather", OP.bypass,
                    replica_groups=[list(range(NCORES))],
                    ins=[ag_in.opt()], outs=[ag_out.opt()],
                )

                oT = acts.tile([P, NH, TC], BF, tag="med1", bufs=2)
                for kv8 in range(NKV):
                    kTt = acts.tile([P, SK, P], BF, tag="kTt", bufs=2)
                    for sk in range(SK):
                        kk = scr.tile([P, P], BF, tag="kk", bufs=2)
                        nc.sync.dma_start(kk[:], ag_out[ds(sk * P, P),
                                                        ds(kv8 * HD, HD)])
                        pt = ps_t.tile([P, P], BF, tag="pt")
                        nc.tensor.transpose(pt[:], kk[:], ident[:])
                        nc.vector.tensor_copy(kTt[:, sk, :], pt[:])
                    vt = acts.tile([P, SK, P], BF, tag="vt", bufs=2)
                    for sk in range(SK):
                        nc.sync.dma_start(vt[:, sk, :],
                                          ag_out[ds(sk * P, P),
                                                 ds(NKV * HD + kv8 * HD, HD)])
                    for h in (2 * kv8, 2 * kv8 + 1):
                        attT = scr.tile([P, SK, TC], BF, tag="attT", bufs=1)
                        for j in range(NJ):
                            sc = ps_big.tile([P, S], F32, tag="sc")
                            for n in range(4):
                                nc.tensor.matmul(
                                    sc[:, ds(n * 512, 512)],
                                    lhsT=qT[:, h, ds(j * P, P)],
                                    rhs=kTt[:, ds(n * 4, 4), :],
                                    start=True, stop=True)
                            att = scr.tile([P, S], BF, tag="att", bufs=1)
                            rsum = scr.tile([P, 1], F32, tag="rsum")
                            nc.scalar.activation(att[:], sc[:], AF.Exp,
                                                 scale=SCALE, accum_out=rsum[:])
                            rrec = scr.tile([P, 1], F32, tag="rrec")
                            nc.vector.reciprocal(rrec[:], rsum[:])
                            nc.vector.tensor_scalar_mul(att[:], att[:], rrec[:, :1])
                            for sk in range(SK):
                                pt = ps_t.tile([P, P], BF, tag="pt")
                                nc.tensor.transpose(pt[:], att[:, ds(sk * P, P)],
                                                    ident[:])
                                nc.vector.tensor_copy(attT[:, sk, ds(j * P, P)], pt[:])
                        op_ = ps_mm.tile([P, TC], F32, tag="mm")
                        for sk in range(SK):
                            nc.tensor.matmul(op_[:], lhsT=vt[:, sk, :],
                                             rhs=attT[:, sk, :],
                                             start=(sk == 0), stop=(sk == SK - 1))
                        nc.vector.tensor_copy(oT[:, h, :], op_[:])

                # --- o proj + residual ---
                for n in range(4):
                    wt = sbw.tile([P, KS, 512], BF, tag="wstream")
                    nc.sync.dma_start(wt[:], wo_ap[l][:, :, ds(n * 512, 512)])
                    for jt in range(NJ):
                        pp = ps_mm.tile([P, 512], F32, tag="mm")
                        for ks in range(KS):
                            nc.tensor.matmul(pp[:], lhsT=oT[:, ks, ds(jt * P, P)],
                                             rhs=wt[:, ks, :],
                                             start=(ks == 0), stop=(ks == KS - 1))
                        nc.vector.tensor_add(x[:, jt, ds(n * 512, 512)],
                                             x[:, jt, ds(n * 512, 512)], pp[:])

                # ======== MLP ========
                # I split into 4 quarters of 2048 to bound SBUF for the
                # silu(g)*u tensor; down-proj accumulates partials into x.
                h2T = rmsnorm_T()
                QD = DS // 4  # 16 I-subtiles per quarter
                for qtr in range(4):
                    act_sb = acts.tile([P, QD, TC], BF, tag="act_sb", bufs=1)
                    for icq in range(QD):
                        ic = qtr * QD + icq
                        wgt = sbw3.tile([P, KS, P], BF, tag="wg")
                        nc.sync.dma_start(wgt[:], wg_ap[l][:, ic, :, :])
                        wut = sbw3.tile([P, KS, P], BF, tag="wu")
                        nc.sync.dma_start(wut[:], wu_ap[l][:, ic, :, :])
                        gp = ps_mm.tile([P, TC], F32, tag="mm")
                        for ks in range(KS):
                            nc.tensor.matmul(gp[:], lhsT=wgt[:, ks, :],
                                             rhs=h2T[:, ks, :],
                                             start=(ks == 0), stop=(ks == KS - 1))
                        up = ps_big.tile([P, TC], F32, tag="sc")
                        for ks in range(KS):
                            nc.tensor.matmul(up[:], lhsT=wut[:, ks, :],
                                             rhs=h2T[:, ks, :],
                                             start=(ks == 0), stop=(ks == KS - 1))
                        gs = scr.tile([P, TC], F32, tag="gs")
                        nc.scalar.activation(gs[:], gp[:], AF.Silu)
                        nc.vector.tensor_tensor(act_sb[:, icq, :], gs[:], up[:],
                                                OP.mult)
                    # partial down proj for this quarter
                    for n in range(8):
                        wt = sbw.tile([P, QD, 256], BF, tag="wstream")
                        nc.sync.dma_start(
                            wt[:], wd_ap[l][:, ds(qtr * QD, QD), ds(n * 256, 256)])
                        for jt in range(NJ):
                            pp = ps_mm.tile([P, 256], F32, tag="mm")
                            for ks in range(QD):
                                nc.tensor.matmul(pp[:],
                                                 lhsT=act_sb[:, ks, ds(jt * P, P)],
                                                 rhs=wt[:, ks, :],
                                                 start=(ks == 0),
                                                 stop=(ks == QD - 1))
                            nc.vector.tensor_add(x[:, jt, ds(n * 256, 256)],
                                                 x[:, jt, ds(n * 256, 256)], pp[:])

            # ---- final rmsnorm ----
            for j in range(NJ):
                sq = ps_big.tile([P, S], F32, tag="sc")
                ssum = scr.tile([P, 1], F32, tag="ssum")
                nc.scalar.activation(sq[:], x[:, j, :], AF.Square, accum_out=ssum[:])
                var = scr.tile([P, 1], F32, tag="var")
                nc.vector.tensor_scalar(var[:], ssum[:], 1.0 / H, EPS, OP.mult, OP.add)
                rec = scr.tile([P, 1], F32, tag="rec")
                nc.vector.reciprocal(rec[:], var[:])
                rstd = scr.tile([P, 1], F32, tag="rstd")
                nc.scalar.activation(rstd[:], rec[:], AF.Sqrt)
                fin = scr.tile([P, H], F32, tag="fin", bufs=1)
                nc.vector.tensor_scalar_mul(fin[:], x[:, j, :], rstd[:, :1])
                nc.vector.tensor_tensor(fin[:], fin[:], nw_sb[:], OP.mult)
                nc.sync.dma_start(out_ap[ds(j * P, P), :], fin[:])

    nc.compile()
    return nc


def _prep_inputs(input_ids, embed, Wq, Wk, Wv, Wo, Wg, Wu, Wd, ln1, ln2, norm_w):
    bf16 = ml_dtypes.bfloat16
    f32 = np.float32
    ids = np.asarray(input_ids).reshape(S).astype(np.int32)
    embed = np.ascontiguousarray(np.asarray(embed, f32).astype(bf16))

    ln1 = np.asarray(ln1, f32)
    ln2 = np.asarray(ln2, f32)

    def kmajor(w, ks):
        w = np.asarray(w, f32)
        Ld, K, N = w.shape
        return np.ascontiguousarray(
            w.reshape(Ld, ks, P, N).transpose(0, 2, 1, 3).astype(bf16))

    wq = kmajor(np.asarray(Wq, f32) * ln1[:, :, None], KS)
    wk = kmajor(np.asarray(Wk, f32) * ln1[:, :, None], KS)
    wv = kmajor(np.asarray(Wv, f32) * ln1[:, :, None], KS)
    wo = kmajor(np.asarray(Wo, f32), KS)
    wd = kmajor(np.asarray(Wd, f32), DS)

    def gmajor(w):
        w = np.asarray(w, f32) * ln2[:, :, None]
        # [L, H, I] -> [L, kp=128, ic=64, ks=16, col=128]
        return np.ascontiguousarray(
            w.reshape(L, KS, P, DS, P).transpose(0, 2, 3, 1, 4).astype(bf16))

    wg = gmajor(Wg)
    wu = gmajor(Wu)

    inv = 1.0 / (THETA ** (np.arange(0, HD, 2, dtype=np.float64) / HD))  # [64]
    pos = np.arange(S, dtype=np.float64)
    fr = pos[:, None] * inv[None, :]
    cosf = np.concatenate([np.cos(fr), np.cos(fr)], -1).astype(f32)  # [S, 128]
    sinf = np.concatenate([np.sin(fr), np.sin(fr)], -1).astype(f32)

    nw = np.ascontiguousarray(np.tile(np.asarray(norm_w, f32)[None, :], (P, 1)))

    in_maps = []
    for c in range(NCORES):
        lo = c * TC
        cc = np.ascontiguousarray(
            cosf[lo:lo + TC].reshape(NJ, P, HD).transpose(1, 0, 2))
        sc_ = np.ascontiguousarray(
            sinf[lo:lo + TC].reshape(NJ, P, HD).transpose(1, 0, 2))
        in_maps.append(dict(
            ids=ids[lo:lo + TC].copy(), embed=embed,
            wq=wq, wk=wk, wv=wv, wo=wo, wg=wg, wu=wu, wd=wd,
            cos=cc, sin=sc_, normw=nw,
        ))
    return in_maps


LAST_NC = None
LAST_IN_MAPS = None


def kernel(**inputs):
    global LAST_RESULT, LAST_NC, LAST_IN_MAPS
    in_maps = _prep_inputs(**inputs)
    nc = _build()
    res = bass_utils.run_bass_kernel_spmd(nc, in_maps, core_ids=list(range(NCORES)))
    LAST_RESULT = res
    LAST_NC = nc
    LAST_IN_MAPS = in_maps
    out = np.concatenate([np.asarray(res.results[c]["out"], np.float32)
                          for c in range(NCORES)], axis=0)
    return out.reshape(B, S, H)
